# revision 24
# baseline (speedup 1.0000x reference)
"""Multi-head attention forward (B=8, S=1024, H=16, D=64) on 8 TRN2 NeuronCores.

Sharding: pure data-parallel over batch - core b computes batch element b
end-to-end (QKV projections + 16-head attention). Zero collectives.

v4 structure (all exact 16-bit math - fp8 fails the 2e-2 gate because ctx is
a near-uniform softmax average ~30x smaller than V, which amplifies any
quantization on the Et/V path by the same factor):
  - x loads ride the HWDGE (sync) queue in f32 while the weights stream
    through SWDGE (gpsimd, casting f32->fp16) in a hand-ordered column-group
    sequence; the two queues share HBM so the first exp issues ~40us in.
  - per x chunk: ScalarE (idle during the load phase) casts f32->fp16, PE
    transposes at 1 cyc/row, DVE evacuates PSUM->xT.
  - fp16 everywhere 16-bit (same PE/DVE/ScalarE speed as bf16, ~16x less
    quantization error; rel err ~1e-3 total, wide margin under the gate).
  - V is computed in natural [s, d] layout (lhsT = x_to^T slice, rhs = Wv
    columns) - no V' transposes; the V bias is added on the PSUM->SBUF
    evacuation as a tensor_tensor add against a partition-broadcast copy of
    bv (no bias matmuls); the fp16 ones column in V' makes the softmax
    denominator fall out of the ctx matmul for free.
  - the pair loop is paced by the ScalarE exp stream (1.1us per 1024-wide
    tile, 142us total): between each pair of score tiles a thunk queue
    drains ~4us of deferred PE work (prev pair's ctx + back-end, next
    pair's K proj, V chunks, late Q projs) so neither engine starves.
  - back-end: ctx'^T tiles are PE-transposed 4-at-a-time into one PSUM tile,
    one strided reciprocal gives 4 denominators, and a single stride-0
    broadcast tensor_tensor multiply normalizes 4x64 outputs at once.
"""

import numpy as np
from contextlib import ExitStack

import concourse.bass as bass
import concourse.mybir as mybir
import concourse.tile as tile
from concourse import bacc
from concourse.masks import make_identity
from concourse.bass_utils import run_bass_kernel_spmd

B, S, H, D = 8, 1024, 16, 64
W = H * D  # 1024
P = 128
N_CORES = 8
F32 = mybir.dt.float32
F16 = mybir.dt.float16
AF = mybir.ActivationFunctionType
ALU = mybir.AluOpType

ST = S // P   # 8 s-tiles
KT_ = W // P  # 8 contraction tiles
IH = 2        # 512-wide halves of the moving dim
HD1 = D + 1   # 65: V' width per head (ones column -> softmax denominator)
NP = H // 2   # 8 head pairs
WG = 256      # weight-load column-group width
XC = 8        # x chunks per tensor (1 s-tile each)


def _dedup_ldweights(nc):
    """Drop InstLdweights that reload the exact weights already resident in
    the PE array. Runs post-compile, so syncs are final: only duplicates with
    empty sync_info, separated from the previous load purely by matmuls on
    the PE stream, are removed."""
    removed = 0
    for f in nc.m.functions:
        for blk in f.blocks:
            ins = blk.instructions
            last_key = None
            to_remove = []
            for i in ins:
                if str(getattr(i, "engine", None)) != "EngineType.PE":
                    continue
                tn = type(i).__name__
                if tn == "InstLdweights":
                    si = i.sync_info
                    clean = si is None or (not si.on_wait and not si.on_update)
                    key = (str(i.ins), str(getattr(i, "is_transpose", None)),
                           str(getattr(i, "tile_position", None)),
                           str(getattr(i, "perf_mode", None)))
                    if clean and key == last_key:
                        to_remove.append(i)
                    else:
                        last_key = key
                elif tn != "InstMatmult":
                    last_key = None
            for i in to_remove:
                ins.remove(i)
            removed += len(to_remove)
    return removed


def build_kernel():
    nc = bacc.Bacc(trn_type="TRN2", target_bir_lowering=False, debug=False,
                   num_devices=N_CORES)

    xf_ext = nc.dram_tensor("from_tensor", [S, W], F32, kind="ExternalInput").ap()
    xt_ext = nc.dram_tensor("to_tensor", [S, W], F32, kind="ExternalInput").ap()
    wq_ext = nc.dram_tensor("Wq", [W, W], F32, kind="ExternalInput").ap()
    bq_ext = nc.dram_tensor("bq", [W], F32, kind="ExternalInput").ap()
    wk_ext = nc.dram_tensor("Wk", [W, W], F32, kind="ExternalInput").ap()
    bk_ext = nc.dram_tensor("bk", [W], F32, kind="ExternalInput").ap()
    wv_ext = nc.dram_tensor("Wv", [W, W], F32, kind="ExternalInput").ap()
    bv_ext = nc.dram_tensor("bv", [W], F32, kind="ExternalInput").ap()
    out_ext = nc.dram_tensor("out", [S, W], F32, kind="ExternalOutput").ap()

    with tile.TileContext(nc) as tc, ExitStack() as top:
        const = top.enter_context(tc.tile_pool(name="const", bufs=1))
        big = top.enter_context(tc.tile_pool(name="big", bufs=1))
        work = top.enter_context(tc.tile_pool(name="work", bufs=4, space="PSUM"))
        pss_pool = top.enter_context(
            tc.tile_pool(name="pss", bufs=2, space="PSUM"))

        ident = const.tile([P, P], F16, tag="ident")
        make_identity(nc, ident[:])
        # preload the exp table set (~2.7us) off the critical path
        scratch = const.tile([P, 8], F32, tag="scratch")
        nc.vector.memset(scratch[:], 0.0)
        nc.scalar.activation(scratch[:, 0:4], scratch[:, 4:8], AF.Exp)

        # ---- persistent SBUF tensors ----
        xTf = big.tile([P, KT_ * S], F16, tag="xTf")   # xT[p, kt*S+s]
        xTt = big.tile([P, KT_ * S], F16, tag="xTt")
        wq_all = big.tile([P, KT_ * W], F16, tag="wq")  # w[p, kt*W+f]
        wk_all = big.tile([P, KT_ * W], F16, tag="wk")
        wv_all = big.tile([P, KT_ * W], F16, tag="wv")
        QT_all = big.tile([P, NP * S], F16, tag="QT")   # [hh*64+d, mt*S+s]
        KT_all = big.tile([P, NP * S], F16, tag="KT")
        # V natural layout + ones col: Vnat[p, st*H*65 + h*65 + d]
        Vnat = big.tile([P, ST * H * HD1], F16, tag="Vnat")
        nc.vector.memset(
            Vnat[:].rearrange("p (t h c) -> p t h c", h=H, c=HD1)[:, :, :, D:HD1],
            1.0)

        bq_sb = const.tile([P, KT_], F32, tag="bq")
        bk_sb = const.tile([P, KT_], F32, tag="bk")
        bv_bc = const.tile([P, W], F16, tag="bv_bc")  # bv on every partition

        # ---------- weight load stream (SWDGE queue, casting f32->f16) ----
        def load_w_grp(dst, src, g):
            nc.gpsimd.dma_start(
                dst.rearrange("p (t f) -> p t f", f=W)[:, :, g * WG:(g + 1) * WG],
                src.rearrange("(t p) f -> p t f", p=P)[:, :, g * WG:(g + 1) * WG])

        nc.gpsimd.dma_start(bq_sb[:], bq_ext.rearrange("(t p) -> p t", p=P))
        nc.gpsimd.dma_start(bk_sb[:], bk_ext.rearrange("(t p) -> p t", p=P))
        nc.gpsimd.dma_start(
            bv_bc[:], bv_ext.rearrange("(a w) -> a w", a=1).broadcast_to([P, W]))
        load_w_grp(wq_all, wq_ext, 0)
        load_w_grp(wq_all, wq_ext, 1)
        load_w_grp(wq_all, wq_ext, 2)
        load_w_grp(wq_all, wq_ext, 3)
        load_w_grp(wk_all, wk_ext, 0)
        load_w_grp(wk_all, wk_ext, 1)
        load_w_grp(wv_all, wv_ext, 0)
        load_w_grp(wv_all, wv_ext, 1)
        load_w_grp(wk_all, wk_ext, 2)
        load_w_grp(wk_all, wk_ext, 3)
        load_w_grp(wv_all, wv_ext, 2)
        load_w_grp(wv_all, wv_ext, 3)

        sm_pool = top.enter_context(tc.tile_pool(name="sm", bufs=2))
        rv_pool = top.enter_context(tc.tile_pool(name="rv", bufs=4))
        out_pool = top.enter_context(tc.tile_pool(name="outp", bufs=2))

        # ---------- x load stream (HWDGE sync queue, f32) ----
        ph0 = ExitStack()
        xr_pool = ph0.enter_context(tc.tile_pool(name="xr", bufs=2))
        xc_pool = ph0.enter_context(tc.tile_pool(name="xc", bufs=2))
        xcf = {}
        xct = {}

        def load_x_chunk(store, x_ext, ch, gate=None):
            xr = xr_pool.tile([P, W], F32, tag="xr", name=f"xr{ch}")
            if gate is not None:
                # tiny WAW-dep write: holds this DMA (and the rest of the
                # FIFO sync queue) until `gate`'s source has landed, so the
                # Wq stream gets the HBM bandwidth first and the Q
                # projections can run back-to-back through the ramp
                nc.vector.tensor_copy(xr[0:1, 0:8], gate)
            nc.sync.dma_start(
                xr[:], x_ext.rearrange("(t p) f -> p t f", p=P)[:, ch, :])
            store[ch] = xr

        for ch in range(XC):
            load_x_chunk(xcf, xf_ext, ch)
        for ch in range(XC):
            load_x_chunk(xct, xt_ext, ch,
                         gate=wq_all[0:1, 900:908] if ch == 0 else None)

        # ---------- PE work emitters ----------
        def transpose_chunk(xr, xT_all, ch):
            """ScalarE casts f32->fp16, PE transposes, DVE evacuates."""
            xc = xc_pool.tile([P, W], F16, tag="xc", name="xc")
            nc.scalar.copy(xc[:], xr[:])
            for wt in range(KT_):
                pt = work.tile([P, P], F16, tag="work", name="pt")
                nc.tensor.transpose(
                    pt[:], xc[:, wt * P: wt * P + P], ident[:])
                nc.vector.tensor_copy(
                    xT_all[:, wt * S + ch * P: wt * S + (ch + 1) * P],
                    pt[:])

        def proj_quarter(dstT, w_all, xT_all, b_sb, mt, q, ps):
            """Quarter of a Q/K projection: kt in [2q, 2q+2) x both halves.
            ps is the pair of PSUM tiles held across the 4 quarters."""
            for kt in range(2 * q, 2 * q + 2):
                for ih in range(IH):
                    nc.tensor.matmul(
                        ps[ih][:],
                        lhsT=w_all[:, kt * W + mt * P: kt * W + mt * P + P],
                        rhs=xT_all[:, kt * S + ih * 512: kt * S + (ih + 1) * 512],
                        start=(kt == 0), stop=(kt == KT_ - 1))
            if q == 3:
                for ih in range(IH):
                    nc.vector.tensor_scalar_add(
                        dstT[:, mt * S + ih * 512: mt * S + (ih + 1) * 512],
                        ps[ih][:], b_sb[:, mt:mt + 1])

        def proj_pair(dstT, w_all, xT_all, b_sb, mt):
            ps = [work.tile([P, 512], F32, tag="work", name=f"pp{ih}")
                  for ih in range(IH)]
            for q in range(4):
                proj_quarter(dstT, w_all, xT_all, b_sb, mt, q, ps)

        def v_chunk(g, st):
            """V projection in natural layout for s-tile st, columns
            [g*512, (g+1)*512) (heads g*8 .. g*8+7); bias added on the
            evacuation via the partition-broadcast bv copy."""
            vps = work.tile([P, 512], F32, tag="work", name="vps")
            for kt in range(KT_):
                nc.tensor.matmul(
                    vps[:],
                    lhsT=xTt[:, kt * S + st * P: kt * S + (st + 1) * P],
                    rhs=wv_all[:, kt * W + g * 512: kt * W + (g + 1) * 512],
                    start=(kt == 0), stop=(kt == KT_ - 1))
            dst = Vnat[:].rearrange("p (t h c) -> p t h c", h=H, c=HD1)[
                :, st, g * 8:(g + 1) * 8, 0:D]
            nc.vector.tensor_tensor(
                dst, vps[:].rearrange("p (h c) -> p h c", c=D),
                bv_bc[:, g * 512:(g + 1) * 512].rearrange(
                    "p (h c) -> p h c", c=D),
                ALU.add)

        Et = {}       # (pair, jt, hh) -> fp16 tile [P, S]
        out_ps = {}   # pair -> out_p tile
        ctxb_s = {}   # (pair, hh) -> ctxb tile

        def scores_tile(p, jt, heads=(0, 1)):
            """scores^T + exp for pair p, s-tile jt: per head, two N=512
            matmuls (shared LDWEIGHTS) into a 2-bank fp32 PSUM tile, then one
            1024-wide exp on ScalarE. The two heads' matmuls hit disjoint PE
            row groups so they pack."""
            for hh in heads:
                pss = pss_pool.tile([P, S], F32, tag="pss", name="pss")
                for ih in range(IH):
                    nc.tensor.matmul(
                        pss[:, ih * 512:(ih + 1) * 512],
                        lhsT=KT_all[hh * D:(hh + 1) * D,
                                    p * S + jt * P: p * S + jt * P + P],
                        rhs=QT_all[hh * D:(hh + 1) * D,
                                   p * S + ih * 512: p * S + (ih + 1) * 512],
                        start=True, stop=True)
                et = et_pool.tile([P, S], F16, tag="et", name="et")
                nc.scalar.activation(et[:], pss[:], AF.Exp, scale=0.125)
                Et[(p, jt, hh)] = et

        def ctx_quarter(p, hh, q, pc_box):
            """Quarter of a ctx half: jt in [2q, 2q+2); q==3 also
            evacuates to ctxb."""
            if q == 0:
                pc_box['pc'] = [
                    work.tile([HD1, 512], F32, tag="work", name=f"pc{ih}")
                    for ih in range(IH)]
            pc = pc_box['pc']
            for jt in range(2 * q, 2 * q + 2):
                for ih in range(IH):
                    nc.tensor.matmul(
                        pc[ih][:],
                        lhsT=Vnat[:, jt * H * HD1 + (2 * p + hh) * HD1:
                                  jt * H * HD1 + (2 * p + hh + 1) * HD1],
                        rhs=Et[(p, jt, hh)][:, ih * 512:(ih + 1) * 512],
                        start=(jt == 0), stop=(jt == ST - 1))
            if q == 3:
                ctxb = sm_pool.tile([HD1, S], F16, tag="ctxb", name="ctxb")
                for ih in range(IH):
                    nc.vector.tensor_copy(ctxb[:, ih * 512:(ih + 1) * 512],
                                          pc[ih][:])
                ctxb_s[(p, hh)] = ctxb

        def ctx_half(p, hh):
            box = {}
            for q in range(4):
                ctx_quarter(p, hh, q, box)

        def backend_quarter(p, hh, self_g):
            """transpose ctx'^T back to [i, d] 4 s-tiles at a time, one
            strided reciprocal + one broadcast multiply per group."""
            if hh == 0 and self_g == 0:
                out_p = out_pool.tile([P, ST * P], F32, tag="outp",
                                      name="out_p")
                out_ps[p] = out_p
            else:
                out_p = out_ps[p]
            ctxb = ctxb_s[(p, hh)]
            for g in (self_g,):
                po = work.tile([P, 4 * 72], F16, tag="work", name="po")
                for k in range(4):
                    it = g * 4 + k
                    nc.tensor.transpose(
                        po[:, k * 72: k * 72 + HD1],
                        ctxb[:, it * P:(it + 1) * P],
                        ident[0:HD1, 0:HD1])
                po3 = po[:].rearrange("p (g c) -> p g c", c=72)
                rv = rv_pool.tile([P, 4], F32, tag="rv", name="rv")
                nc.vector.reciprocal(rv[:], po3[:, :, D:D + 1])
                dst = out_p[:].rearrange("p (t c) -> p t c", c=P)[
                    :, g * 4:(g + 1) * 4, hh * D:(hh + 1) * D]
                nc.vector.tensor_tensor(
                    dst, po3[:, :, 0:D],
                    rv[:].rearrange("p g -> p g ()").broadcast_to([P, 4, D]),
                    ALU.mult)
            if hh == 1 and self_g == 1:
                ctxb_s.pop((p, 0))
                ctxb_s.pop((p, 1))
                nc.sync.dma_start(
                    out_ext.rearrange("(t p) (g c) -> p t g c", p=P, c=P)[
                        :, :, p, :],
                    out_p.rearrange("p (t c) -> p t c", c=P))

        def backend_half(p, hh):
            backend_quarter(p, hh, 0)
            backend_quarter(p, hh, 1)

        # ---------- phase 0: transposes + early projections ----------
        for ch in range(XC):
            transpose_chunk(xcf[ch], xTf, ch)
        proj_pair(QT_all, wq_all, xTf, bq_sb, 0)
        transpose_chunk(xct[0], xTt, 0)
        transpose_chunk(xct[1], xTt, 1)
        proj_pair(QT_all, wq_all, xTf, bq_sb, 1)
        transpose_chunk(xct[2], xTt, 2)
        transpose_chunk(xct[3], xTt, 3)
        proj_pair(QT_all, wq_all, xTf, bq_sb, 2)
        transpose_chunk(xct[4], xTt, 4)
        transpose_chunk(xct[5], xTt, 5)
        proj_pair(QT_all, wq_all, xTf, bq_sb, 3)
        transpose_chunk(xct[6], xTt, 6)
        transpose_chunk(xct[7], xTt, 7)
        for mt in range(4, NP):
            proj_pair(QT_all, wq_all, xTf, bq_sb, mt)
        proj_pair(KT_all, wk_all, xTt, bk_sb, 0)
        ph0.close()  # frees x-staging SBUF for the Et pool
        et_pool = top.enter_context(tc.tile_pool(name="et", bufs=28))

        # ---------- exp-paced pair loop with a cost-budgeted thunk queue ----
        queue = []  # list of (cost_us, emit_fn)

        def q_proj(dstT, w_all, xT_all, b_sb, mt):
            ps_box = {}
            for q in range(4):
                def emit(q=q, mt=mt):
                    if q == 0:
                        ps_box['ps'] = [
                            work.tile([P, 512], F32, tag="work", name=f"pp{ih}")
                            for ih in range(IH)]
                    proj_quarter(dstT, w_all, xT_all, b_sb, mt, q,
                                 ps_box['ps'])
                queue.append((1.0, emit))
                counts['appended'] += 1

        def q_ctx(p, hh):
            box = {}
            for q in range(4):
                queue.append(
                    (0.9, lambda q=q: ctx_quarter(p, hh, q, box)))
                counts['appended'] += 1

        def q_back(p, hh):
            for g in range(2):
                queue.append(
                    (0.7, lambda g=g: backend_quarter(p, hh, g)))
                counts['appended'] += 1

        def q_v(g, st):
            queue.append((2.0, lambda: v_chunk(g, st)))
            counts['appended'] += 1

        counts = {'appended': 0, 'drained': 0}
        k_mark = {}

        def drain(budget):
            # peek rule: never overshoot the slot budget (a bunched slot
            # starves the exp stream and stalls the PE on the scores PSUM
            # rotation two tiles later)
            spent = 0.0
            while queue and spent + queue[0][0] <= budget:
                cost, emit = queue.pop(0)
                emit()
                counts['drained'] += 1
                spent += cost

        def force_drain_to(mark):
            # correctness invariant: everything appended up to `mark` must be
            # EMITTED before the dependent scores are, else Tile sees a read
            # with no prior writer and orders the projection after the read
            while queue and counts['drained'] < mark:
                cost, emit = queue.pop(0)
                emit()
                counts['drained'] += 1

        for p in range(NP):
            if p + 1 < NP:
                q_proj(KT_all, wk_all, xTt, bk_sb, p + 1)
                k_mark[p + 1] = counts['appended']
            if p == 0:
                for st in range(ST):
                    q_v(0, st)
            if p >= 1:
                q_ctx(p - 1, 0)
                q_ctx(p - 1, 1)
                q_back(p - 1, 0)
                q_back(p - 1, 1)
            if 1 <= p <= 4:
                for st in range(2 * (p - 1), 2 * p):
                    q_v(1, st)
            force_drain_to(k_mark.get(p, 0))
            if p < NP - 1:
                for jtp in range(4):
                    scores_tile(p, 2 * jtp)
                    scores_tile(p, 2 * jtp + 1)
                    drain(5.3)
            else:
                # last pair: emit head 0's scores first so ctx(7, h0) can
                # overlap head 1's exp stream, shortening the tail
                for jtp in range(4):
                    scores_tile(p, 2 * jtp, heads=(0,))
                    scores_tile(p, 2 * jtp + 1, heads=(0,))
                    drain(2.6)
                drain(1e9)
                for jtp in range(4):
                    scores_tile(p, 2 * jtp, heads=(1,))
                    scores_tile(p, 2 * jtp + 1, heads=(1,))
                    if jtp == 2:
                        ctx_half(p, 0)
                    elif jtp == 3:
                        backend_half(p, 0)

        ctx_half(NP - 1, 1)
        backend_half(NP - 1, 1)

    nc.compile()
    _dedup_ldweights(nc)
    return nc


def run(inputs, trace=False, trace_kwargs=None):
    """inputs: dict of full-shape np arrays as in reference.setup_inputs()."""
    nc = build_kernel()
    in_maps = []
    for b in range(N_CORES):
        in_maps.append({
            "from_tensor": np.ascontiguousarray(np.asarray(inputs["from_tensor"][b], dtype=np.float32)),
            "to_tensor": np.ascontiguousarray(np.asarray(inputs["to_tensor"][b], dtype=np.float32)),
            "Wq": np.asarray(inputs["Wq"], dtype=np.float32),
            "bq": np.asarray(inputs["bq"], dtype=np.float32),
            "Wk": np.asarray(inputs["Wk"], dtype=np.float32),
            "bk": np.asarray(inputs["bk"], dtype=np.float32),
            "Wv": np.asarray(inputs["Wv"], dtype=np.float32),
            "bv": np.asarray(inputs["bv"], dtype=np.float32),
        })
    res = run_bass_kernel_spmd(nc, in_maps, core_ids=list(range(N_CORES)),
                               trace=trace, **(trace_kwargs or {}))
    out = np.stack([np.asarray(res.results[b]["out"]) for b in range(N_CORES)],
                   axis=0).astype(np.float32)
    return out, res


def kernel(**inputs):
    out, _ = run(inputs, trace=False)
    return out


# revision 25
# speedup vs baseline: 1.0320x; 1.0320x over previous
"""Multi-head attention forward (B=8, S=1024, H=16, D=64) on 8 TRN2 NeuronCores.

Sharding: pure data-parallel over batch - core b computes batch element b
end-to-end (QKV projections + 16-head attention). Zero collectives.

v4 structure (all exact 16-bit math - fp8 fails the 2e-2 gate because ctx is
a near-uniform softmax average ~30x smaller than V, which amplifies any
quantization on the Et/V path by the same factor):
  - x loads ride the HWDGE (sync) queue in f32 while the weights stream
    through SWDGE (gpsimd, casting f32->fp16) in a hand-ordered column-group
    sequence; the two queues share HBM so the first exp issues ~40us in.
  - per x chunk: ScalarE (idle during the load phase) casts f32->fp16, PE
    transposes at 1 cyc/row, DVE evacuates PSUM->xT.
  - fp16 everywhere 16-bit (same PE/DVE/ScalarE speed as bf16, ~16x less
    quantization error; rel err ~1e-3 total, wide margin under the gate).
  - V is computed in natural [s, d] layout (lhsT = x_to^T slice, rhs = Wv
    columns) - no V' transposes; the V bias is added on the PSUM->SBUF
    evacuation as a tensor_tensor add against a partition-broadcast copy of
    bv (no bias matmuls); the fp16 ones column in V' makes the softmax
    denominator fall out of the ctx matmul for free.
  - the pair loop is paced by the ScalarE exp stream (1.1us per 1024-wide
    tile, 142us total): between each pair of score tiles a thunk queue
    drains ~4us of deferred PE work (prev pair's ctx + back-end, next
    pair's K proj, V chunks, late Q projs) so neither engine starves.
  - back-end: ctx'^T tiles are PE-transposed 4-at-a-time into one PSUM tile,
    one strided reciprocal gives 4 denominators, and a single stride-0
    broadcast tensor_tensor multiply normalizes 4x64 outputs at once.
"""

import numpy as np
from contextlib import ExitStack

import concourse.bass as bass
import concourse.mybir as mybir
import concourse.tile as tile
from concourse import bacc
from concourse.masks import make_identity
from concourse.bass_utils import run_bass_kernel_spmd

B, S, H, D = 8, 1024, 16, 64
W = H * D  # 1024
P = 128
N_CORES = 8
F32 = mybir.dt.float32
F16 = mybir.dt.float16
AF = mybir.ActivationFunctionType
ALU = mybir.AluOpType

ST = S // P   # 8 s-tiles
KT_ = W // P  # 8 contraction tiles
IH = 2        # 512-wide halves of the moving dim
HD1 = D + 1   # 65: V' width per head (ones column -> softmax denominator)
NP = H // 2   # 8 head pairs
WG = 256      # weight-load column-group width
XC = 8        # x chunks per tensor (1 s-tile each)


def _dedup_ldweights(nc):
    """Drop InstLdweights that reload the exact weights already resident in
    the PE array. Runs post-compile, so syncs are final: only duplicates with
    empty sync_info, separated from the previous load purely by matmuls on
    the PE stream, are removed."""
    removed = 0
    for f in nc.m.functions:
        for blk in f.blocks:
            ins = blk.instructions
            last_key = None
            to_remove = []
            for i in ins:
                if str(getattr(i, "engine", None)) != "EngineType.PE":
                    continue
                tn = type(i).__name__
                if tn == "InstLdweights":
                    si = i.sync_info
                    clean = si is None or (not si.on_wait and not si.on_update)
                    key = (str(i.ins), str(getattr(i, "is_transpose", None)),
                           str(getattr(i, "tile_position", None)),
                           str(getattr(i, "perf_mode", None)))
                    if clean and key == last_key:
                        to_remove.append(i)
                    else:
                        last_key = key
                elif tn != "InstMatmult":
                    last_key = None
            for i in to_remove:
                ins.remove(i)
            removed += len(to_remove)
    return removed


def build_kernel():
    nc = bacc.Bacc(trn_type="TRN2", target_bir_lowering=False, debug=False,
                   num_devices=N_CORES)

    xf_ext = nc.dram_tensor("from_tensor", [S, W], F32, kind="ExternalInput").ap()
    xt_ext = nc.dram_tensor("to_tensor", [S, W], F32, kind="ExternalInput").ap()
    wq_ext = nc.dram_tensor("Wq", [W, W], F32, kind="ExternalInput").ap()
    bq_ext = nc.dram_tensor("bq", [W], F32, kind="ExternalInput").ap()
    wk_ext = nc.dram_tensor("Wk", [W, W], F32, kind="ExternalInput").ap()
    bk_ext = nc.dram_tensor("bk", [W], F32, kind="ExternalInput").ap()
    wv_ext = nc.dram_tensor("Wv", [W, W], F32, kind="ExternalInput").ap()
    bv_ext = nc.dram_tensor("bv", [W], F32, kind="ExternalInput").ap()
    out_ext = nc.dram_tensor("out", [S, W], F32, kind="ExternalOutput").ap()

    with tile.TileContext(nc) as tc, ExitStack() as top:
        const = top.enter_context(tc.tile_pool(name="const", bufs=1))
        big = top.enter_context(tc.tile_pool(name="big", bufs=1))
        work = top.enter_context(tc.tile_pool(name="work", bufs=4, space="PSUM"))
        pss_pool = top.enter_context(
            tc.tile_pool(name="pss", bufs=2, space="PSUM"))

        ident = const.tile([P, P], F16, tag="ident")
        make_identity(nc, ident[:])
        # preload the exp table set (~2.7us) off the critical path
        scratch = const.tile([P, 8], F32, tag="scratch")
        nc.vector.memset(scratch[:], 0.0)
        nc.scalar.activation(scratch[:, 0:4], scratch[:, 4:8], AF.Exp)

        # ---- persistent SBUF tensors ----
        xTf = big.tile([P, KT_ * S], F16, tag="xTf")   # xT[p, kt*S+s]
        xTt = big.tile([P, KT_ * S], F16, tag="xTt")
        wq_all = big.tile([P, KT_ * W], F16, tag="wq")  # w[p, kt*W+f]
        wk_all = big.tile([P, KT_ * W], F16, tag="wk")
        wv_all = big.tile([P, KT_ * W], F16, tag="wv")
        QT_all = big.tile([P, NP * S], F16, tag="QT")   # [hh*64+d, mt*S+s]
        KT_all = big.tile([P, NP * S], F16, tag="KT")
        # V natural layout + ones col: Vnat[p, st*H*65 + h*65 + d]
        Vnat = big.tile([P, ST * H * HD1], F16, tag="Vnat")
        nc.vector.memset(
            Vnat[:].rearrange("p (t h c) -> p t h c", h=H, c=HD1)[:, :, :, D:HD1],
            1.0)

        bq_sb = const.tile([P, KT_], F32, tag="bq")
        bk_sb = const.tile([P, KT_], F32, tag="bk")
        bv_bc = const.tile([P, W], F16, tag="bv_bc")  # bv on every partition

        # ---------- weight load stream (SWDGE queue, casting f32->f16) ----
        def load_w_grp(dst, src, g):
            nc.gpsimd.dma_start(
                dst.rearrange("p (t f) -> p t f", f=W)[:, :, g * WG:(g + 1) * WG],
                src.rearrange("(t p) f -> p t f", p=P)[:, :, g * WG:(g + 1) * WG])

        nc.gpsimd.dma_start(bq_sb[:], bq_ext.rearrange("(t p) -> p t", p=P))
        nc.gpsimd.dma_start(bk_sb[:], bk_ext.rearrange("(t p) -> p t", p=P))
        nc.gpsimd.dma_start(
            bv_bc[:], bv_ext.rearrange("(a w) -> a w", a=1).broadcast_to([P, W]))
        load_w_grp(wq_all, wq_ext, 0)
        load_w_grp(wq_all, wq_ext, 1)
        load_w_grp(wq_all, wq_ext, 2)
        load_w_grp(wq_all, wq_ext, 3)
        load_w_grp(wk_all, wk_ext, 0)
        load_w_grp(wk_all, wk_ext, 1)
        load_w_grp(wv_all, wv_ext, 0)
        load_w_grp(wv_all, wv_ext, 1)
        load_w_grp(wk_all, wk_ext, 2)
        load_w_grp(wk_all, wk_ext, 3)
        load_w_grp(wv_all, wv_ext, 2)
        load_w_grp(wv_all, wv_ext, 3)

        sm_pool = top.enter_context(tc.tile_pool(name="sm", bufs=2))
        rv_pool = top.enter_context(tc.tile_pool(name="rv", bufs=4))
        out_pool = top.enter_context(tc.tile_pool(name="outp", bufs=2))

        # ---------- x load stream (HWDGE sync queue, f32) ----
        ph0 = ExitStack()
        xr_pool = ph0.enter_context(tc.tile_pool(name="xr", bufs=2))
        xc_pool = ph0.enter_context(tc.tile_pool(name="xc", bufs=2))
        xcf = {}
        xct = {}

        def load_x_chunk(store, x_ext, ch, gate=None):
            xr = xr_pool.tile([P, W], F32, tag="xr", name=f"xr{ch}")
            if gate is not None:
                # tiny WAW-dep write: holds this DMA (and the rest of the
                # FIFO sync queue) until `gate`'s source has landed, so the
                # Wq stream gets the HBM bandwidth first and the Q
                # projections can run back-to-back through the ramp
                nc.vector.tensor_copy(xr[0:1, 0:8], gate)
            nc.sync.dma_start(
                xr[:], x_ext.rearrange("(t p) f -> p t f", p=P)[:, ch, :])
            store[ch] = xr

        for ch in range(XC):
            load_x_chunk(xcf, xf_ext, ch)
        for ch in range(XC):
            load_x_chunk(xct, xt_ext, ch)

        # ---------- PE work emitters ----------
        def transpose_chunk(xr, xT_all, ch):
            """ScalarE casts f32->fp16, PE transposes, DVE evacuates."""
            xc = xc_pool.tile([P, W], F16, tag="xc", name="xc")
            nc.scalar.copy(xc[:], xr[:])
            for wt in range(KT_):
                pt = work.tile([P, P], F16, tag="work", name="pt")
                nc.tensor.transpose(
                    pt[:], xc[:, wt * P: wt * P + P], ident[:])
                nc.vector.tensor_copy(
                    xT_all[:, wt * S + ch * P: wt * S + (ch + 1) * P],
                    pt[:])

        def proj_quarter(dstT, w_all, xT_all, b_sb, mt, q, ps):
            """Quarter of a Q/K projection: kt in [2q, 2q+2) x both halves.
            ps is the pair of PSUM tiles held across the 4 quarters."""
            for kt in range(2 * q, 2 * q + 2):
                for ih in range(IH):
                    nc.tensor.matmul(
                        ps[ih][:],
                        lhsT=w_all[:, kt * W + mt * P: kt * W + mt * P + P],
                        rhs=xT_all[:, kt * S + ih * 512: kt * S + (ih + 1) * 512],
                        start=(kt == 0), stop=(kt == KT_ - 1))
            if q == 3:
                for ih in range(IH):
                    nc.vector.tensor_scalar_add(
                        dstT[:, mt * S + ih * 512: mt * S + (ih + 1) * 512],
                        ps[ih][:], b_sb[:, mt:mt + 1])

        def proj_pair(dstT, w_all, xT_all, b_sb, mt):
            ps = [work.tile([P, 512], F32, tag="work", name=f"pp{ih}")
                  for ih in range(IH)]
            for q in range(4):
                proj_quarter(dstT, w_all, xT_all, b_sb, mt, q, ps)

        def v_chunk(g, st):
            """V projection in natural layout for s-tile st, columns
            [g*512, (g+1)*512) (heads g*8 .. g*8+7); bias added on the
            evacuation via the partition-broadcast bv copy."""
            vps = work.tile([P, 512], F32, tag="work", name="vps")
            for kt in range(KT_):
                nc.tensor.matmul(
                    vps[:],
                    lhsT=xTt[:, kt * S + st * P: kt * S + (st + 1) * P],
                    rhs=wv_all[:, kt * W + g * 512: kt * W + (g + 1) * 512],
                    start=(kt == 0), stop=(kt == KT_ - 1))
            dst = Vnat[:].rearrange("p (t h c) -> p t h c", h=H, c=HD1)[
                :, st, g * 8:(g + 1) * 8, 0:D]
            nc.vector.tensor_tensor(
                dst, vps[:].rearrange("p (h c) -> p h c", c=D),
                bv_bc[:, g * 512:(g + 1) * 512].rearrange(
                    "p (h c) -> p h c", c=D),
                ALU.add)

        Et = {}       # (pair, jt, hh) -> fp16 tile [P, S]
        out_ps = {}   # pair -> out_p tile
        ctxb_s = {}   # (pair, hh) -> ctxb tile

        def scores_tile(p, jt, heads=(0, 1)):
            """scores^T + exp for pair p, s-tile jt: per head, two N=512
            matmuls (shared LDWEIGHTS) into a 2-bank fp32 PSUM tile, then one
            1024-wide exp on ScalarE. The two heads' matmuls hit disjoint PE
            row groups so they pack."""
            for hh in heads:
                pss = pss_pool.tile([P, S], F32, tag="pss", name="pss")
                for ih in range(IH):
                    nc.tensor.matmul(
                        pss[:, ih * 512:(ih + 1) * 512],
                        lhsT=KT_all[hh * D:(hh + 1) * D,
                                    p * S + jt * P: p * S + jt * P + P],
                        rhs=QT_all[hh * D:(hh + 1) * D,
                                   p * S + ih * 512: p * S + (ih + 1) * 512],
                        start=True, stop=True)
                et = et_pool.tile([P, S], F16, tag="et", name="et")
                nc.scalar.activation(et[:], pss[:], AF.Exp, scale=0.125)
                Et[(p, jt, hh)] = et

        def ctx_quarter(p, hh, q, pc_box):
            """Quarter of a ctx half: jt in [2q, 2q+2); q==3 also
            evacuates to ctxb."""
            if q == 0:
                pc_box['pc'] = [
                    work.tile([HD1, 512], F32, tag="work", name=f"pc{ih}")
                    for ih in range(IH)]
            pc = pc_box['pc']
            for jt in range(2 * q, 2 * q + 2):
                for ih in range(IH):
                    nc.tensor.matmul(
                        pc[ih][:],
                        lhsT=Vnat[:, jt * H * HD1 + (2 * p + hh) * HD1:
                                  jt * H * HD1 + (2 * p + hh + 1) * HD1],
                        rhs=Et[(p, jt, hh)][:, ih * 512:(ih + 1) * 512],
                        start=(jt == 0), stop=(jt == ST - 1))
            if q == 3:
                ctxb = sm_pool.tile([HD1, S], F16, tag="ctxb", name="ctxb")
                for ih in range(IH):
                    nc.vector.tensor_copy(ctxb[:, ih * 512:(ih + 1) * 512],
                                          pc[ih][:])
                ctxb_s[(p, hh)] = ctxb

        def ctx_half(p, hh):
            box = {}
            for q in range(4):
                ctx_quarter(p, hh, q, box)

        def backend_quarter(p, hh, self_g):
            """transpose ctx'^T back to [i, d] 4 s-tiles at a time, one
            strided reciprocal + one broadcast multiply per group."""
            if hh == 0 and self_g == 0:
                out_p = out_pool.tile([P, ST * P], F32, tag="outp",
                                      name="out_p")
                out_ps[p] = out_p
            else:
                out_p = out_ps[p]
            ctxb = ctxb_s[(p, hh)]
            for g in (self_g,):
                po = work.tile([P, 4 * 72], F16, tag="work", name="po")
                for k in range(4):
                    it = g * 4 + k
                    nc.tensor.transpose(
                        po[:, k * 72: k * 72 + HD1],
                        ctxb[:, it * P:(it + 1) * P],
                        ident[0:HD1, 0:HD1])
                po3 = po[:].rearrange("p (g c) -> p g c", c=72)
                rv = rv_pool.tile([P, 4], F32, tag="rv", name="rv")
                nc.vector.reciprocal(rv[:], po3[:, :, D:D + 1])
                dst = out_p[:].rearrange("p (t c) -> p t c", c=P)[
                    :, g * 4:(g + 1) * 4, hh * D:(hh + 1) * D]
                nc.vector.tensor_tensor(
                    dst, po3[:, :, 0:D],
                    rv[:].rearrange("p g -> p g ()").broadcast_to([P, 4, D]),
                    ALU.mult)
            if hh == 1 and self_g == 1:
                ctxb_s.pop((p, 0))
                ctxb_s.pop((p, 1))
                nc.sync.dma_start(
                    out_ext.rearrange("(t p) (g c) -> p t g c", p=P, c=P)[
                        :, :, p, :],
                    out_p.rearrange("p (t c) -> p t c", c=P))

        def backend_half(p, hh):
            backend_quarter(p, hh, 0)
            backend_quarter(p, hh, 1)

        # ---------- phase 0: transposes + early projections ----------
        for ch in range(XC):
            transpose_chunk(xcf[ch], xTf, ch)
        proj_pair(QT_all, wq_all, xTf, bq_sb, 0)
        transpose_chunk(xct[0], xTt, 0)
        transpose_chunk(xct[1], xTt, 1)
        proj_pair(QT_all, wq_all, xTf, bq_sb, 1)
        transpose_chunk(xct[2], xTt, 2)
        transpose_chunk(xct[3], xTt, 3)
        proj_pair(QT_all, wq_all, xTf, bq_sb, 2)
        transpose_chunk(xct[4], xTt, 4)
        transpose_chunk(xct[5], xTt, 5)
        proj_pair(QT_all, wq_all, xTf, bq_sb, 3)
        transpose_chunk(xct[6], xTt, 6)
        transpose_chunk(xct[7], xTt, 7)
        for mt in range(4, NP):
            proj_pair(QT_all, wq_all, xTf, bq_sb, mt)
        proj_pair(KT_all, wk_all, xTt, bk_sb, 0)
        ph0.close()  # frees x-staging SBUF for the Et pool
        et_pool = top.enter_context(tc.tile_pool(name="et", bufs=28))

        # ---------- exp-paced pair loop with a cost-budgeted thunk queue ----
        queue = []  # list of (cost_us, emit_fn)

        def q_proj(dstT, w_all, xT_all, b_sb, mt):
            ps_box = {}
            for q in range(4):
                def emit(q=q, mt=mt):
                    if q == 0:
                        ps_box['ps'] = [
                            work.tile([P, 512], F32, tag="work", name=f"pp{ih}")
                            for ih in range(IH)]
                    proj_quarter(dstT, w_all, xT_all, b_sb, mt, q,
                                 ps_box['ps'])
                queue.append((1.0, emit))
                counts['appended'] += 1

        def q_ctx(p, hh):
            box = {}
            for q in range(4):
                queue.append(
                    (0.9, lambda q=q: ctx_quarter(p, hh, q, box)))
                counts['appended'] += 1

        def q_back(p, hh):
            for g in range(2):
                queue.append(
                    (0.7, lambda g=g: backend_quarter(p, hh, g)))
                counts['appended'] += 1

        def q_v(g, st):
            queue.append((2.0, lambda: v_chunk(g, st)))
            counts['appended'] += 1

        counts = {'appended': 0, 'drained': 0}
        k_mark = {}

        def drain(budget):
            # peek rule: never overshoot the slot budget (a bunched slot
            # starves the exp stream and stalls the PE on the scores PSUM
            # rotation two tiles later)
            spent = 0.0
            while queue and spent + queue[0][0] <= budget:
                cost, emit = queue.pop(0)
                emit()
                counts['drained'] += 1
                spent += cost

        def force_drain_to(mark):
            # correctness invariant: everything appended up to `mark` must be
            # EMITTED before the dependent scores are, else Tile sees a read
            # with no prior writer and orders the projection after the read
            while queue and counts['drained'] < mark:
                cost, emit = queue.pop(0)
                emit()
                counts['drained'] += 1

        for p in range(NP):
            if p + 1 < NP:
                q_proj(KT_all, wk_all, xTt, bk_sb, p + 1)
                k_mark[p + 1] = counts['appended']
            if p == 0:
                for st in range(ST):
                    q_v(0, st)
            if p >= 1:
                q_ctx(p - 1, 0)
                q_ctx(p - 1, 1)
                q_back(p - 1, 0)
                q_back(p - 1, 1)
            if 1 <= p <= 4:
                for st in range(2 * (p - 1), 2 * p):
                    q_v(1, st)
            force_drain_to(k_mark.get(p, 0))
            if p < NP - 1:
                for jtp in range(4):
                    scores_tile(p, 2 * jtp)
                    scores_tile(p, 2 * jtp + 1)
                    drain(5.3)
            else:
                # last pair: emit head 0's scores first so ctx(7, h0) can
                # overlap head 1's exp stream, shortening the tail
                for jtp in range(4):
                    scores_tile(p, 2 * jtp, heads=(0,))
                    scores_tile(p, 2 * jtp + 1, heads=(0,))
                    drain(2.6)
                drain(1e9)
                for jtp in range(4):
                    scores_tile(p, 2 * jtp, heads=(1,))
                    scores_tile(p, 2 * jtp + 1, heads=(1,))
                    if jtp == 2:
                        ctx_half(p, 0)
                    elif jtp == 3:
                        backend_half(p, 0)

        ctx_half(NP - 1, 1)
        backend_half(NP - 1, 1)

    nc.compile()
    _dedup_ldweights(nc)
    return nc


def run(inputs, trace=False, trace_kwargs=None):
    """inputs: dict of full-shape np arrays as in reference.setup_inputs()."""
    nc = build_kernel()
    in_maps = []
    for b in range(N_CORES):
        in_maps.append({
            "from_tensor": np.ascontiguousarray(np.asarray(inputs["from_tensor"][b], dtype=np.float32)),
            "to_tensor": np.ascontiguousarray(np.asarray(inputs["to_tensor"][b], dtype=np.float32)),
            "Wq": np.asarray(inputs["Wq"], dtype=np.float32),
            "bq": np.asarray(inputs["bq"], dtype=np.float32),
            "Wk": np.asarray(inputs["Wk"], dtype=np.float32),
            "bk": np.asarray(inputs["bk"], dtype=np.float32),
            "Wv": np.asarray(inputs["Wv"], dtype=np.float32),
            "bv": np.asarray(inputs["bv"], dtype=np.float32),
        })
    res = run_bass_kernel_spmd(nc, in_maps, core_ids=list(range(N_CORES)),
                               trace=trace, **(trace_kwargs or {}))
    out = np.stack([np.asarray(res.results[b]["out"]) for b in range(N_CORES)],
                   axis=0).astype(np.float32)
    return out, res


def kernel(**inputs):
    out, _ = run(inputs, trace=False)
    return out
